# revision 4
# baseline (speedup 1.0000x reference)
"""GCN message-passing kernel for Trainium2 (8 NeuronCores, Bass/Tile).

Strategy (dest-sharded, gather-based):
  - 200k nodes split into 8 shards of 25k destination rows (one per core).
  - out = node_norm(gcn + fnn) where gcn = (S @ x) @ W_gcn + b_gcn: the GCN
    aggregation commutes with the weight matmul, so each core gathers raw x
    rows for its edges (dma_gather, int16 indices over 8 source windows of
    25k rows), scales by the per-edge norm, and scatter-adds (dma_scatter_add)
    into a per-core HBM accumulator. Duplicate destinations within a scatter
    call race the RMW, so edges are reordered into "rounds" with unique
    destinations per call (host-side lexsort); calls are WAW-serialized.
  - Self loops fold in as agg2 = agg + dinv2*x (host supplies (dinv2*x)^T).
  - Phase C runs feature-major so biases are per-partition: g^T = Wg^T@agg2^T,
    f^T = Lrelu(Wf^T@x^T + bf), s^T = (g^T+bg)+f^T; PE transposes s back and
    ones-matmuls give row sums of s and s^2 for the node-norm.
Inputs are replicated per core (full x for gathers + per-core x^T slices).
"""

import numpy as np

N_SRC = 100000
N_TAR = 100000
N = N_SRC + N_TAR
D = 128
NCORE = 8
SH = N // NCORE          # 25000 dest rows per core
NWIN = 8                 # source windows of SH rows (int16 index limit)
HALF = 12544             # dest rows per half (2*HALF = 25088 >= SH)
SPAD = 2 * HALF          # padded per-core row count (196 tiles of 128)
NT_HALF = HALF // 128    # 98 tiles per half
TW = 2                   # row-tiles per phase-C iteration
EPS = 1e-6
NEG = 0.01

_CACHE = {}
PHASE_B = True          # debug bisect flags
PHASE_C = True


def _wrap16(a):
    w = a.reshape(-1, 16).T
    return np.tile(w, (8, 1)).astype(np.int16, copy=True)


def _wrap128(a):
    return np.ascontiguousarray(a.reshape(-1, 128).T.astype(np.float32))


def _host_prep(x, edge_index, edge_weight):
    """Bucket, dest-sort, round-split and pad all edges. Returns per-core
    per-(half,window) wrapped index/dest/norm arrays + shared round layout."""
    row = np.asarray(edge_index[0], dtype=np.int64)
    col = np.asarray(edge_index[1], dtype=np.int64)
    w = np.asarray(edge_weight, dtype=np.float32)

    deg = np.bincount(col, weights=w.astype(np.float64), minlength=N)
    deg = (deg + 1.0).astype(np.float32)
    dinv = (1.0 / np.sqrt(deg)).astype(np.float32)
    norm = dinv[row] * w * dinv[col]

    core = col // SH
    dstl = col - core * SH
    half = dstl // HALF
    dsth = (dstl - half * HALF).astype(np.int64)
    win = row // SH
    idxl = (row - win * SH).astype(np.int16)

    bucket = (core * 2 + half) * NWIN + win          # 0..127
    # group by (bucket, dest); rank within each (bucket,dest) run = round
    order1 = np.lexsort((dsth, bucket))
    bs, ds = bucket[order1], dsth[order1]
    comb = bs * 16384 + ds
    change = np.empty(len(comb), dtype=bool)
    change[0] = True
    change[1:] = comb[1:] != comb[:-1]
    runstart = np.maximum.accumulate(np.where(change, np.arange(len(comb)), 0))
    rnd = np.arange(len(comb)) - runstart
    order2 = np.lexsort((ds, rnd, bs))               # (bucket, round, dest)
    perm = order1[order2]
    bs2, rnd2 = bucket[perm], rnd[order2]

    nr = int(rnd.max()) + 1
    # per (bucket, round) counts -> padded sizes shared across cores
    cnt = np.bincount(bs2 * nr + rnd2, minlength=128 * nr).reshape(128, nr)
    cnt = cnt.reshape(NCORE, 2 * NWIN, nr)
    pad = ((cnt.max(axis=0) + 127) // 128) * 128      # [2*NWIN, nr]
    rounds = []                                       # per (h,s): [(start,cnt)]
    for hs in range(2 * NWIN):
        r_list, start = [], 0
        for r in range(nr):
            c = int(pad[hs, r])
            if c == 0:
                continue
            r_list.append((start, c))
            start += c
        rounds.append((r_list, start))                # start == B[hs]

    # slice per-core data and pad
    idx_s, dst_s, nrm_s = idxl[perm], dsth[perm], norm[perm]
    off = np.zeros(128 * nr + 1, dtype=np.int64)
    np.cumsum(np.bincount(bs2 * nr + rnd2, minlength=128 * nr), out=off[1:])

    per_core = []
    for k in range(NCORE):
        bufs = {}
        for hs in range(2 * NWIN):
            r_list, B = rounds[hs]
            ii = np.zeros(B, dtype=np.int16)
            dd = np.zeros(B, dtype=np.int64)
            nn_ = np.zeros(B, dtype=np.float32)
            # pads: index 0 (harmless read), dest -> dump rows, norm 0
            dd[:] = HALF + (np.arange(B) % 128)
            b_id = (k * 2 * NWIN) + hs
            pos = 0
            for r_i, (start, c) in enumerate(r_list):
                lo, hi = off[b_id * nr + r_i], off[b_id * nr + r_i + 1]
                n_real = hi - lo
                ii[start:start + n_real] = idx_s[lo:hi]
                dd[start:start + n_real] = dst_s[lo:hi]
                nn_[start:start + n_real] = nrm_s[lo:hi]
                pos += c
            bufs[hs] = (_wrap16(ii), _wrap16(dd.astype(np.int16)), _wrap128(nn_))
        per_core.append(bufs)
    return per_core, rounds, dinv


def _build_program(rounds):
    from concourse import bacc, mybir, tile

    f32 = mybir.dt.float32
    i16 = mybir.dt.int16
    nc = bacc.Bacc(None, num_swdge_queues=2)

    xfull = nc.dram_tensor("xfull", [N, D], f32, kind="ExternalInput")
    xT = nc.dram_tensor("xT", [D, SPAD], f32, kind="ExternalInput")
    xdT = nc.dram_tensor("xdT", [D, SPAD], f32, kind="ExternalInput")
    Wg_d = nc.dram_tensor("Wg", [D, D], f32, kind="ExternalInput")
    Wf_d = nc.dram_tensor("Wf", [D, D], f32, kind="ExternalInput")
    bg_d = nc.dram_tensor("bg", [D, 1], f32, kind="ExternalInput")
    bf_d = nc.dram_tensor("bf", [D, 1], f32, kind="ExternalInput")
    id_d = nc.dram_tensor("ident", [D, D], f32, kind="ExternalInput")
    on_d = nc.dram_tensor("ones", [D, 1], f32, kind="ExternalInput")
    ep_d = nc.dram_tensor("eps", [D, 1], f32, kind="ExternalInput")
    zf_d = nc.dram_tensor("zfill", [128, 2048], f32, kind="ExternalInput")
    meta = {}
    for hs in range(2 * NWIN):
        _, B = rounds[hs]
        meta[hs] = (
            nc.dram_tensor(f"idx{hs}", [128, B // 16], i16, kind="ExternalInput"),
            nc.dram_tensor(f"dst{hs}", [128, B // 16], i16, kind="ExternalInput"),
            nc.dram_tensor(f"nrm{hs}", [128, B // 128], f32, kind="ExternalInput"),
        )
    agg = [nc.dram_tensor(f"agg{h}", [HALF + 128, D], f32) for h in range(2)]
    out_d = nc.dram_tensor("out", [SH, D], f32, kind="ExternalOutput")

    AOp = mybir.AluOpType
    AF = mybir.ActivationFunctionType

    with tile.TileContext(nc) as tc:
        with tc.tile_pool(name="const", bufs=1) as cpool, \
             tc.tile_pool(name="metap", bufs=4) as mpool, \
             tc.tile_pool(name="gath", bufs=3) as gpool, \
             tc.tile_pool(name="work", bufs=3) as wpool, \
             tc.tile_pool(name="psum", bufs=2, space="PSUM") as ppool:

            Wg_t = cpool.tile([D, D], f32, tag="wg")
            Wf_t = cpool.tile([D, D], f32, tag="wf")
            bg_t = cpool.tile([D, 1], f32, tag="bg")
            bf_t = cpool.tile([D, 1], f32, tag="bf")
            id_t = cpool.tile([D, D], f32, tag="id")
            on_t = cpool.tile([D, 1], f32, tag="on")
            ep_t = cpool.tile([D, 1], f32, tag="ep")
            zf_t = cpool.tile([128, 2048], f32, tag="zf")
            for t, d in [(Wg_t, Wg_d), (Wf_t, Wf_d), (bg_t, bg_d), (bf_t, bf_d),
                         (id_t, id_d), (on_t, on_d), (ep_t, ep_d), (zf_t, zf_d)]:
                nc.sync.dma_start(out=t[:], in_=d[:])

            # zero-fill both agg halves (content of zf tile is zeros)
            for h in range(2):
                r0 = 0
                while r0 < HALF + 128:
                    nr_ = min(2048, HALF + 128 - r0)
                    nc.sync.dma_start(out=agg[h][r0:r0 + nr_, :],
                                      in_=zf_t[:, :nr_])
                    r0 += nr_

            # ---- Phase B: gather / scale / scatter per (half, window) ----
            for h in range(2 if PHASE_B else 0):
                for s in range(NWIN):
                    hs = h * NWIN + s
                    r_list, B = rounds[hs]
                    B128 = B // 128
                    idx_t = mpool.tile([128, B // 16], i16, tag="idx")
                    dst_t = mpool.tile([128, B // 16], i16, tag="dst")
                    nrm_t = mpool.tile([128, B128], f32, tag="nrm")
                    nc.sync.dma_start(out=idx_t[:], in_=meta[hs][0][:])
                    nc.sync.dma_start(out=dst_t[:], in_=meta[hs][1][:])
                    nc.sync.dma_start(out=nrm_t[:], in_=meta[hs][2][:])
                    g_t = gpool.tile([128, B128, D], f32, tag="g")
                    nc.gpsimd.dma_gather(
                        out_ap=g_t[:], in_ap=xfull[s * SH:(s + 1) * SH, :],
                        idxs_ap=idx_t[:], num_idxs=B, num_idxs_reg=B,
                        elem_size=D, queue_num=0, single_packet=False)
                    nc.vector.tensor_mul(
                        g_t[:], g_t[:], nrm_t[:].to_broadcast((128, B128, D)))
                    for (start, c) in r_list:
                        nc.gpsimd.dma_scatter_add(
                            out_ap=agg[h][:], idxs_ap=dst_t[:, start // 16:(start + c) // 16],
                            in_ap=g_t[:, start // 128:(start + c) // 128, :],
                            num_idxs=c, num_idxs_reg=c, elem_size=D,
                            queue_num=1, single_packet=False)

            # ---- Phase C: per 2-tile group ----
            W = TW * 128
            for h in range(2 if PHASE_C else 0):
                for ti in range(0, NT_HALF, TW):
                    c0 = h * HALF + ti * 128          # global padded row/col
                    A_t = wpool.tile([128, TW, D], f32, tag="A")
                    for j in range(TW):
                        nc.sync.dma_start(
                            out=A_t[:, j, :],
                            in_=agg[h][ti * 128 + j * 128: ti * 128 + (j + 1) * 128, :])
                    xT_t = wpool.tile([D, W], f32, tag="xT")
                    xdT_t = wpool.tile([D, W], f32, tag="xdT")
                    nc.sync.dma_start(out=xT_t[:], in_=xT[:, c0:c0 + W])
                    nc.sync.dma_start(out=xdT_t[:], in_=xdT[:, c0:c0 + W])

                    ATp = ppool.tile([D, W], f32, tag="ATp")
                    for j in range(TW):
                        nc.tensor.transpose(ATp[:, j * 128:(j + 1) * 128],
                                            A_t[:, j, :], id_t[:])
                    ATs = wpool.tile([D, W], f32, tag="ATs")
                    nc.scalar.copy(out=ATs[:], in_=ATp[:])
                    A2T = wpool.tile([D, W], f32, tag="A2T")
                    nc.vector.tensor_add(A2T[:], ATs[:], xdT_t[:])

                    GpT = ppool.tile([D, W], f32, tag="GpT")
                    nc.tensor.matmul(GpT[:], Wg_t[:], A2T[:], start=True, stop=True)
                    FpT = ppool.tile([D, W], f32, tag="FpT")
                    nc.tensor.matmul(FpT[:], Wf_t[:], xT_t[:], start=True, stop=True)
                    fT = wpool.tile([D, W], f32, tag="fT")
                    nc.scalar.activation(fT[:], FpT[:], AF.Lrelu,
                                         bias=bf_t[:], scale=1.0, alpha=NEG)
                    sT = wpool.tile([D, W], f32, tag="sT")
                    nc.vector.scalar_tensor_tensor(sT[:], GpT[:], bg_t[:], fT[:],
                                                   op0=AOp.add, op1=AOp.add)
                    sqT = wpool.tile([D, W], f32, tag="sqT")
                    nc.scalar.activation(sqT[:], sT[:], AF.Square)

                    # transpose back + row sums (disjoint regions of one bank)
                    s_aug = ppool.tile([D, W + 2 * TW], f32, tag="s_aug")
                    for j in range(TW):
                        sl = sT[:, j * 128:(j + 1) * 128]
                        ql = sqT[:, j * 128:(j + 1) * 128]
                        nc.tensor.transpose(s_aug[:, j * 128:(j + 1) * 128], sl, id_t[:])
                        nc.tensor.matmul(s_aug[:, W + j:W + j + 1], sl, on_t[:],
                                         start=True, stop=True)
                        nc.tensor.matmul(s_aug[:, W + TW + j:W + TW + j + 1], ql,
                                         on_t[:], start=True, stop=True)

                    for j in range(TW):
                        mean_t = wpool.tile([D, 1], f32, tag="mean")
                        nc.vector.tensor_scalar_mul(mean_t[:], s_aug[:, W + j:W + j + 1],
                                                    1.0 / D)
                        msq = wpool.tile([D, 1], f32, tag="msq")
                        nc.vector.tensor_mul(msq[:], mean_t[:], mean_t[:])
                        veps = wpool.tile([D, 1], f32, tag="veps")
                        nc.vector.scalar_tensor_tensor(
                            veps[:], s_aug[:, W + TW + j:W + TW + j + 1], 1.0 / D,
                            msq[:], op0=AOp.mult, op1=AOp.subtract)
                        std = wpool.tile([D, 1], f32, tag="std")
                        nc.scalar.activation(std[:], veps[:], AF.Sqrt, bias=ep_t[:])
                        rstd = wpool.tile([D, 1], f32, tag="rstd")
                        nc.vector.reciprocal(rstd[:], std[:])
                        o1 = wpool.tile([D, D], f32, tag="o1")
                        nc.vector.tensor_scalar(o1[:], s_aug[:, j * 128:(j + 1) * 128],
                                                mean_t[:], rstd[:],
                                                op0=AOp.subtract, op1=AOp.mult)
                        o2 = wpool.tile([D, D], f32, tag="o2")
                        nc.scalar.activation(o2[:], o1[:], AF.Lrelu, alpha=NEG)
                        rg = c0 + j * 128             # padded row id
                        n_out = min(128, max(0, SH - rg))
                        if n_out > 0:
                            nc.sync.dma_start(out=out_d[rg:rg + n_out, :],
                                              in_=o2[:n_out, :])
            if not PHASE_C:
                dummy = cpool.tile([128, 128], f32, tag="dummy")
                nc.vector.memset(dummy[:], 0.0)
                nc.sync.dma_start(out=out_d[0:128, :], in_=dummy[:])
    nc.finalize()
    return nc


def _plan(x_src, x_tar, edge_index, edge_weight, W_gcn, b_gcn, W_fnn, b_fnn):
    """Host prep + (cached) program build. Returns (nc, in_maps, assemble)."""
    x = np.concatenate([np.asarray(x_src, np.float32),
                        np.asarray(x_tar, np.float32)], axis=0)
    per_core, rounds, dinv = _host_prep(x, edge_index, edge_weight)

    key = (PHASE_B, PHASE_C) + tuple(B for (_r, B) in rounds) + tuple(
        tuple(r) for (r, _B) in rounds)
    if key not in _CACHE:
        _CACHE[key] = _build_program(rounds)
    nc = _CACHE[key]

    ident = np.eye(D, dtype=np.float32)
    common = {
        "Wg": np.asarray(W_gcn, np.float32),
        "Wf": np.asarray(W_fnn, np.float32),
        "bg": np.asarray(b_gcn, np.float32).reshape(D, 1),
        "bf": np.asarray(b_fnn, np.float32).reshape(D, 1),
        "ident": ident,
        "ones": np.ones((D, 1), np.float32),
        "eps": np.full((D, 1), EPS, np.float32),
        "zfill": np.zeros((128, 2048), np.float32),
    }
    in_maps = []
    d2 = (dinv * dinv).astype(np.float32)
    for k in range(NCORE):
        xo = x[k * SH:(k + 1) * SH]
        xT_k = np.zeros((D, SPAD), np.float32)
        xT_k[:, :SH] = xo.T
        xdT_k = np.zeros((D, SPAD), np.float32)
        xdT_k[:, :SH] = (xo * d2[k * SH:(k + 1) * SH, None]).T
        m = dict(common)
        m["xfull"] = x
        m["xT"] = xT_k
        m["xdT"] = xdT_k
        for hs in range(2 * NWIN):
            ii, dd, nn_ = per_core[k][hs]
            m[f"idx{hs}"] = ii
            m[f"dst{hs}"] = dd
            m[f"nrm{hs}"] = nn_
        in_maps.append(m)

    def assemble(results):
        full = np.concatenate([results[k]["out"] for k in range(NCORE)], axis=0)
        return full[:N_SRC, :], full[N_SRC:, :]

    return nc, in_maps, assemble


def kernel(x_src, x_tar, edge_index, edge_weight, W_gcn, b_gcn, W_fnn, b_fnn):
    from concourse.bass_utils import run_bass_kernel_spmd

    nc, in_maps, assemble = _plan(x_src, x_tar, edge_index, edge_weight,
                                  W_gcn, b_gcn, W_fnn, b_fnn)
    res = run_bass_kernel_spmd(nc, in_maps, list(range(NCORE)))
    return assemble(res.results)
